# revision 48
# baseline (speedup 1.0000x reference)
"""BoxFilter kernel for Trainium2 (8 NeuronCores), bf16 I/O.

Computes out[b,0,i,j] = sum_{c} sum_{|di|<=15} sum_{|dj|<=15} x[b,c,i+di,j+dj]
(edge-clamped 31x31 box filter over the channel-summed image), matching the
reference cumsum + shifted-diff formulation (separable box sums).

The correctness gate is rel_err < 2e-2; bf16 end-to-end measures ~5e-3, so
all HBM traffic runs in bf16 (half the bytes of the f32 baseline):
host casts x to bf16 and interleaves channels per row ([S_ROWS, C, W] per
core), the device computes with bf16 SBUF tiles + f32 PSUM/scan state, and
the output returns as bf16 which the host upcasts.

Sharding: data-parallel over (batch, H-half) -> 8 shards, no cross-core
communication. Each core gets a host-padded [1056, 3, 2048] slab (16 halo
rows each side, zero-filled past the global image edges).

Per-core pipeline (per 128-row output tile):
  1. one contiguous 1.5MB DMA -> xc[128, 3, 2048] (HWDGE, alternating rings)
  2. channel sum: 2 tensor_tensor adds, bf16 (DVE 2x-packed) or GpSimd,
     split per-tile to balance the engines
  3. vertical 31-tap box sum: two banded 0/1 bf16 matmuls per PSUM bank
  4. ACT copies PSUM f32 -> bf16 xp tile (pads pre-zeroed once)
  5. horizontal 31-tap box sum: DVE tensor_tensor_scan (fp32 state,
     bf16 in/out): state_j = state_{j-1} + xp[j] - xp[j-31]
  6. DMA box rows (bf16) to DRAM
"""

import numpy as np
import ml_dtypes

BF16 = ml_dtypes.bfloat16

R = 15
TAP = 2 * R + 1          # 31
B, C, H, W = 4, 3, 2048, 2048
HALF = H // 2            # 1024 output rows per core
S_ROWS = HALF + 32       # 1056 input rows per core (16-row halo each side)
N_CORES = 8
PAD_L = TAP              # left zero pad for the scan (31)
PAD_R = R                # right zero pad (15)
XP_W = PAD_L + W + PAD_R # 2094
SCAN_N = W + R           # 2063 scan steps; out col j = scan[j + R]
P = 128                  # SBUF partitions
N_OUT_TILES = HALF // P  # 8
TAIL_ROWS = S_ROWS - N_OUT_TILES * P  # 32 valid rows in the 9th s-tile
MM_N = 512               # one PSUM bank of f32

# Channel sum strategy: ch0+ch1 with ONE DVE tensor_add (bf16 2x-packed),
# ch2 via SWDGE DMA-accumulate straight from HBM into s. GpSimd tensor ops
# are banned from the steady state: a concurrent GpSimd TT slows DVE ops
# 2-4x (shared SBUF port lockout), and the scans live on DVE.

_CACHE = {}


def _band_matrices():
    # out row i of a 128-row tile needs halo'd input rows r = i+1 .. i+31
    # (r is the row index within the [s_lo; s_hi] 256-row window).
    k = np.arange(P)[:, None]
    i = np.arange(P)[None, :]
    band_a = ((k >= i + 1) & (k <= i + TAP)).astype(BF16)          # rows in s_lo
    band_b = ((k + P >= i + 1) & (k + P <= i + TAP)).astype(BF16)  # rows in s_hi
    return band_a, band_b


def _build_kernel(tc, nc, out, xs, bands_d, mybir, bass):
    from contextlib import ExitStack

    bf = mybir.dt.bfloat16
    f32 = mybir.dt.float32
    add = mybir.AluOpType.add
    sub = mybir.AluOpType.subtract

    with ExitStack() as ctx:
        const_pool = ctx.enter_context(tc.tile_pool(name="const", bufs=1))
        xc_pool = ctx.enter_context(tc.tile_pool(name="xc", bufs=6))
        xcf_pool = ctx.enter_context(tc.tile_pool(name="xcf", bufs=4))
        s_pool = ctx.enter_context(tc.tile_pool(name="s", bufs=6))
        xp_pool = ctx.enter_context(tc.tile_pool(name="xp", bufs=3))
        box_pool = ctx.enter_context(tc.tile_pool(name="box", bufs=4))
        psum_pool = ctx.enter_context(
            tc.tile_pool(name="psum", bufs=8, space=bass.MemorySpace.PSUM)
        )

        # both bands in one [128, 256] load: 512B per partition line keeps
        # the descriptors at the SDMA line-rate minimum (separate [128,128]
        # loads are 256B/line and crawl through the RMW path)
        bands = const_pool.tile([P, 2 * P], bf)
        band_a = bands[:, 0:P]
        band_b = bands[:, P : 2 * P]

        # xp buffers: zero the scan pads once; ACT only ever writes the
        # middle [PAD_L, PAD_L+W) region.
        xp_tiles = [
            xp_pool.tile([P, XP_W], bf, name=f"xp{i}") for i in range(3)
        ]
        for xp in xp_tiles:
            nc.gpsimd.memset(xp[:, 0:PAD_L], 0.0)
            nc.gpsimd.memset(xp[:, PAD_L + W : XP_W], 0.0)

        # bands go on the otherwise-idle SWDGE queue: a small-descriptor
        # transfer at an HWDGE ring head delays that ring's spin-up and every
        # input tile behind it. (Also: one combined 512B/partition transfer,
        # not two 256B ones - sub-512B descriptors take the slow RMW path.)
        nc.gpsimd.dma_start(bands[:], bands_d)

        # tiles 0..FILL-1 load all 3 channels and sum on DVE (which has slack
        # during the pipeline fill); later tiles fold ch2 onto the prefetched
        # ch0 with an SWDGE accumulate at load time - by then the accum queue
        # has tiles of slack before the matmuls need the result
        FILL = 4

        def load_xc(u):
            rows = P if u < N_OUT_TILES else TAIL_ROWS
            nch = 3 if u < FILL else 2
            pool = xcf_pool if u < FILL else xc_pool
            xc = pool.tile([P, nch, W], bf, name="xcf" if u < FILL else "xc")
            # bulk input rides the sync ring; tile 1 takes the (still empty)
            # scalar ring so s_0 and s_1 land in parallel at fill time
            eng = nc.scalar if u == 1 else nc.sync
            eng.dma_start(xc[:rows], xs[P * u : P * u + rows, 0:nch, :])
            if u >= FILL:
                nc.gpsimd.dma_start(
                    xc[:rows, 0, :],
                    xs[P * u : P * u + rows, 2, :],
                    accum_op=add,
                    single_packet=True,
                )
            return xc

        def make_s(u, xc):
            rows = P if u < N_OUT_TILES else TAIL_ROWS
            s = s_pool.tile([P, W], bf)
            if rows < P:
                # rows past the slab are multiplied by zero band weights but
                # must be finite.
                nc.gpsimd.memset(s[:], 0.0)
            nc.vector.tensor_add(s[:rows, :], xc[:rows, 0, :], xc[:rows, 1, :])
            if u < FILL:
                nc.vector.tensor_add(s[:rows, :], s[:rows, :], xc[:rows, 2, :])
            return s

        # DMA prefetch runs several tiles ahead; the channel-sum adds for
        # tile t+2 are issued AFTER scan(t) so that on the in-order DVE
        # queue every scan runs as soon as its data is ready instead of
        # sitting behind adds that wait on far-future DMAs.
        DMA_AHEAD = 6
        xcs = {u: load_xc(u) for u in range(DMA_AHEAD)}
        s_tiles = {u: make_s(u, xcs.pop(u)) for u in range(2)}
        for t in range(N_OUT_TILES):
            u = t + DMA_AHEAD
            if u <= N_OUT_TILES:
                xcs[u] = load_xc(u)
            s_lo, s_hi = s_tiles.pop(t), s_tiles[t + 1]

            xp = xp_tiles[t % 3]

            # all band_a matmuls, then all band_b: minimizes PE weight reloads
            psums = []
            for nb in range(W // MM_N):
                ps = psum_pool.tile([P, MM_N], f32)
                lo_c = s_lo[:, MM_N * nb : MM_N * (nb + 1)]
                nc.tensor.matmul(ps[:], band_a[:], lo_c, start=True, stop=False)
                psums.append(ps)
            for nb in range(W // MM_N):
                hi_c = s_hi[:, MM_N * nb : MM_N * (nb + 1)]
                nc.tensor.matmul(
                    psums[nb][:], band_b[:], hi_c, start=False, stop=True
                )
                nc.scalar.copy(
                    xp[:, PAD_L + MM_N * nb : PAD_L + MM_N * (nb + 1)],
                    psums[nb][:],
                )

            box = box_pool.tile([P, SCAN_N + 1], bf)
            nc.vector.tensor_tensor_scan(
                box[:, 0:SCAN_N],
                xp[:, PAD_L : PAD_L + SCAN_N],
                xp[:, 0:SCAN_N],
                0.0,
                add,
                sub,
            )
            if t + 2 <= N_OUT_TILES:
                s_tiles[t + 2] = make_s(t + 2, xcs.pop(t + 2))
            # even stores on scalar; odd on sync (free once input drains)
            store_eng = nc.scalar if t % 2 == 0 else nc.sync
            store_eng.dma_start(out[P * t : P * (t + 1), :], box[:, R : R + W])


def _get_nc():
    if "nc" in _CACHE:
        return _CACHE["nc"]
    import concourse.bass as bass
    import concourse.tile as tile
    from concourse import bacc, mybir

    nc = bacc.Bacc(
        "TRN2", target_bir_lowering=False, debug=False, num_devices=N_CORES
    )
    xs = nc.dram_tensor("xs", [S_ROWS, C, W], mybir.dt.bfloat16, kind="ExternalInput")
    bd = nc.dram_tensor("bands", [P, 2 * P], mybir.dt.bfloat16, kind="ExternalInput")
    out = nc.dram_tensor("out", [HALF, W], mybir.dt.bfloat16, kind="ExternalOutput")

    with tile.TileContext(nc) as tc:
        _build_kernel(tc, nc, out.ap(), xs.ap(), bd.ap(), mybir, bass)
    nc.compile()
    _CACHE["nc"] = nc
    return nc


def _in_maps(x):
    band_a, band_b = _band_matrices()
    bands = np.concatenate([band_a, band_b], axis=1)
    xb = x.astype(BF16)
    maps = []
    for k in range(N_CORES):
        b, half = divmod(k, 2)
        h0 = half * HALF
        lo = h0 - 16  # global row of xs row 0
        g0, g1 = max(lo, 0), min(h0 + HALF + 16, H)
        xs = np.zeros((S_ROWS, C, W), BF16)
        # [C, rows, W] -> [rows, C, W]
        xs[g0 - lo : g1 - lo] = xb[b, :, g0:g1, :].transpose(1, 0, 2)
        maps.append({"xs": xs, "bands": bands})
    return maps


def _run(x, trace=False, tmpdir=None):
    from concourse.bass_utils import run_bass_kernel_spmd

    nc = _get_nc()
    res = run_bass_kernel_spmd(
        nc, _in_maps(x), list(range(N_CORES)), trace=trace, tmpdir=tmpdir
    )
    out = np.empty((B, 1, H, W), np.float32)
    for k in range(N_CORES):
        b, half = divmod(k, 2)
        out[b, 0, half * HALF : (half + 1) * HALF, :] = (
            res.results[k]["out"].astype(np.float32)
        )
    return out, res


def kernel(x: np.ndarray) -> np.ndarray:
    x = np.ascontiguousarray(x, dtype=np.float32)
    assert x.shape == (B, C, H, W)
    return _run(x)[0]


# revision 50
# speedup vs baseline: 1.0413x; 1.0413x over previous
"""BoxFilter kernel for Trainium2 (8 NeuronCores), bf16 I/O.

Computes out[b,0,i,j] = sum_{c} sum_{|di|<=15} sum_{|dj|<=15} x[b,c,i+di,j+dj]
(edge-clamped 31x31 box filter over the channel-summed image), matching the
reference cumsum + shifted-diff formulation (separable box sums).

The correctness gate is rel_err < 2e-2; bf16 end-to-end measures ~5e-3, so
all HBM traffic runs in bf16 (half the bytes of the f32 baseline):
host casts x to bf16 and interleaves channels per row ([S_ROWS, C, W] per
core), the device computes with bf16 SBUF tiles + f32 PSUM/scan state, and
the output returns as bf16 which the host upcasts.

Sharding: data-parallel over (batch, H-half) -> 8 shards, no cross-core
communication. Each core gets a host-padded [1056, 3, 2048] slab (16 halo
rows each side, zero-filled past the global image edges).

Per-core pipeline (per 128-row output tile):
  1. one contiguous 1.5MB DMA -> xc[128, 3, 2048] (HWDGE, alternating rings)
  2. channel sum: 2 tensor_tensor adds, bf16 (DVE 2x-packed) or GpSimd,
     split per-tile to balance the engines
  3. vertical 31-tap box sum: two banded 0/1 bf16 matmuls per PSUM bank
  4. ACT copies PSUM f32 -> bf16 xp tile (pads pre-zeroed once)
  5. horizontal 31-tap box sum: DVE tensor_tensor_scan (fp32 state,
     bf16 in/out): state_j = state_{j-1} + xp[j] - xp[j-31]
  6. DMA box rows (bf16) to DRAM
"""

import numpy as np
import ml_dtypes

BF16 = ml_dtypes.bfloat16

R = 15
TAP = 2 * R + 1          # 31
B, C, H, W = 4, 3, 2048, 2048
HALF = H // 2            # 1024 output rows per core
S_ROWS = HALF + 32       # 1056 input rows per core (16-row halo each side)
N_CORES = 8
PAD_L = TAP              # left zero pad for the scan (31)
PAD_R = R                # right zero pad (15)
XP_W = PAD_L + W + PAD_R # 2094
SCAN_N = W + R           # 2063 scan steps; out col j = scan[j + R]
P = 128                  # SBUF partitions
N_OUT_TILES = HALF // P  # 8
TAIL_ROWS = S_ROWS - N_OUT_TILES * P  # 32 valid rows in the 9th s-tile
MM_N = 512               # one PSUM bank of f32

# Channel sum strategy: ch0+ch1 with ONE DVE tensor_add (bf16 2x-packed),
# ch2 via SWDGE DMA-accumulate straight from HBM into s. GpSimd tensor ops
# are banned from the steady state: a concurrent GpSimd TT slows DVE ops
# 2-4x (shared SBUF port lockout), and the scans live on DVE.

_CACHE = {}


def _band_matrices():
    # out row i of a 128-row tile needs halo'd input rows r = i+1 .. i+31
    # (r is the row index within the [s_lo; s_hi] 256-row window).
    k = np.arange(P)[:, None]
    i = np.arange(P)[None, :]
    band_a = ((k >= i + 1) & (k <= i + TAP)).astype(BF16)          # rows in s_lo
    band_b = ((k + P >= i + 1) & (k + P <= i + TAP)).astype(BF16)  # rows in s_hi
    return band_a, band_b


def _build_kernel(tc, nc, out, xs, bands_d, mybir, bass):
    from contextlib import ExitStack

    bf = mybir.dt.bfloat16
    f32 = mybir.dt.float32
    add = mybir.AluOpType.add
    sub = mybir.AluOpType.subtract

    with ExitStack() as ctx:
        const_pool = ctx.enter_context(tc.tile_pool(name="const", bufs=1))
        xc_pool = ctx.enter_context(tc.tile_pool(name="xc", bufs=6))
        xcf_pool = ctx.enter_context(tc.tile_pool(name="xcf", bufs=4))
        s_pool = ctx.enter_context(tc.tile_pool(name="s", bufs=6))
        xp_pool = ctx.enter_context(tc.tile_pool(name="xp", bufs=3))
        box_pool = ctx.enter_context(tc.tile_pool(name="box", bufs=4))
        psum_pool = ctx.enter_context(
            tc.tile_pool(name="psum", bufs=8, space=bass.MemorySpace.PSUM)
        )

        # both bands in one [128, 256] load: 512B per partition line keeps
        # the descriptors at the SDMA line-rate minimum (separate [128,128]
        # loads are 256B/line and crawl through the RMW path)
        bands = const_pool.tile([P, 2 * P], bf)
        band_a = bands[:, 0:P]
        band_b = bands[:, P : 2 * P]

        # xp buffers: zero the scan pads once; ACT only ever writes the
        # middle [PAD_L, PAD_L+W) region.
        xp_tiles = [
            xp_pool.tile([P, XP_W], bf, name=f"xp{i}") for i in range(3)
        ]
        for xp in xp_tiles:
            nc.gpsimd.memset(xp[:, 0:PAD_L], 0.0)
            nc.gpsimd.memset(xp[:, PAD_L + W : XP_W], 0.0)

        # bands go on the otherwise-idle SWDGE queue: a small-descriptor
        # transfer at an HWDGE ring head delays that ring's spin-up and every
        # input tile behind it. (Also: one combined 512B/partition transfer,
        # not two 256B ones - sub-512B descriptors take the slow RMW path.)
        nc.gpsimd.dma_start(bands[:], bands_d)

        # tiles 0..FILL-1 load all 3 channels and sum on DVE (which has slack
        # during the pipeline fill); later tiles fold ch2 onto the prefetched
        # ch0 with an SWDGE accumulate at load time - by then the accum queue
        # has tiles of slack before the matmuls need the result
        FILL = 3

        def load_xc(u):
            rows = P if u < N_OUT_TILES else TAIL_ROWS
            nch = 3 if u < FILL else 2
            pool = xcf_pool if u < FILL else xc_pool
            xc = pool.tile([P, nch, W], bf, name="xcf" if u < FILL else "xc")
            # bulk input rides the sync ring; tile 1 takes the (still empty)
            # scalar ring so s_0 and s_1 land in parallel at fill time
            eng = nc.scalar if u == 1 else nc.sync
            eng.dma_start(xc[:rows], xs[P * u : P * u + rows, 0:nch, :])
            return xc

        def make_s(u, xc):
            rows = P if u < N_OUT_TILES else TAIL_ROWS
            s = s_pool.tile([P, W], bf)
            if rows < P:
                # rows past the slab are multiplied by zero band weights but
                # must be finite.
                nc.gpsimd.memset(s[:], 0.0)
            nc.vector.tensor_add(s[:rows, :], xc[:rows, 0, :], xc[:rows, 1, :])
            if u < FILL:
                # fill-phase tiles: DVE has slack, and the early accum queue
                # is packet-starved behind the input ring
                nc.vector.tensor_add(s[:rows, :], s[:rows, :], xc[:rows, 2, :])
            else:
                nc.gpsimd.dma_start(
                    s[:rows, :],
                    xs[P * u : P * u + rows, 2, :],
                    accum_op=add,
                    single_packet=True,
                )
            return s

        # DMA prefetch runs several tiles ahead; the channel-sum adds for
        # tile t+2 are issued AFTER scan(t) so that on the in-order DVE
        # queue every scan runs as soon as its data is ready instead of
        # sitting behind adds that wait on far-future DMAs.
        DMA_AHEAD = 6
        xcs = {u: load_xc(u) for u in range(DMA_AHEAD)}
        s_tiles = {u: make_s(u, xcs.pop(u)) for u in range(2)}
        for t in range(N_OUT_TILES):
            u = t + DMA_AHEAD
            if u <= N_OUT_TILES:
                xcs[u] = load_xc(u)
            s_lo, s_hi = s_tiles.pop(t), s_tiles[t + 1]

            xp = xp_tiles[t % 3]

            # all band_a matmuls, then all band_b: minimizes PE weight reloads
            psums = []
            for nb in range(W // MM_N):
                ps = psum_pool.tile([P, MM_N], f32)
                lo_c = s_lo[:, MM_N * nb : MM_N * (nb + 1)]
                nc.tensor.matmul(ps[:], band_a[:], lo_c, start=True, stop=False)
                psums.append(ps)
            for nb in range(W // MM_N):
                hi_c = s_hi[:, MM_N * nb : MM_N * (nb + 1)]
                nc.tensor.matmul(
                    psums[nb][:], band_b[:], hi_c, start=False, stop=True
                )
                nc.scalar.copy(
                    xp[:, PAD_L + MM_N * nb : PAD_L + MM_N * (nb + 1)],
                    psums[nb][:],
                )

            box = box_pool.tile([P, SCAN_N + 1], bf)
            nc.vector.tensor_tensor_scan(
                box[:, 0:SCAN_N],
                xp[:, PAD_L : PAD_L + SCAN_N],
                xp[:, 0:SCAN_N],
                0.0,
                add,
                sub,
            )
            if t + 2 <= N_OUT_TILES:
                s_tiles[t + 2] = make_s(t + 2, xcs.pop(t + 2))
            # even stores on scalar; odd on sync (free once input drains)
            store_eng = nc.scalar if t % 2 == 0 else nc.sync
            store_eng.dma_start(out[P * t : P * (t + 1), :], box[:, R : R + W])


def _get_nc():
    if "nc" in _CACHE:
        return _CACHE["nc"]
    import concourse.bass as bass
    import concourse.tile as tile
    from concourse import bacc, mybir

    nc = bacc.Bacc(
        "TRN2", target_bir_lowering=False, debug=False, num_devices=N_CORES
    )
    xs = nc.dram_tensor("xs", [S_ROWS, C, W], mybir.dt.bfloat16, kind="ExternalInput")
    bd = nc.dram_tensor("bands", [P, 2 * P], mybir.dt.bfloat16, kind="ExternalInput")
    out = nc.dram_tensor("out", [HALF, W], mybir.dt.bfloat16, kind="ExternalOutput")

    with tile.TileContext(nc) as tc:
        _build_kernel(tc, nc, out.ap(), xs.ap(), bd.ap(), mybir, bass)
    nc.compile()
    _CACHE["nc"] = nc
    return nc


def _in_maps(x):
    band_a, band_b = _band_matrices()
    bands = np.concatenate([band_a, band_b], axis=1)
    xb = x.astype(BF16)
    maps = []
    for k in range(N_CORES):
        b, half = divmod(k, 2)
        h0 = half * HALF
        lo = h0 - 16  # global row of xs row 0
        g0, g1 = max(lo, 0), min(h0 + HALF + 16, H)
        xs = np.zeros((S_ROWS, C, W), BF16)
        # [C, rows, W] -> [rows, C, W]
        xs[g0 - lo : g1 - lo] = xb[b, :, g0:g1, :].transpose(1, 0, 2)
        maps.append({"xs": xs, "bands": bands})
    return maps


def _run(x, trace=False, tmpdir=None):
    from concourse.bass_utils import run_bass_kernel_spmd

    nc = _get_nc()
    res = run_bass_kernel_spmd(
        nc, _in_maps(x), list(range(N_CORES)), trace=trace, tmpdir=tmpdir
    )
    out = np.empty((B, 1, H, W), np.float32)
    for k in range(N_CORES):
        b, half = divmod(k, 2)
        out[b, 0, half * HALF : (half + 1) * HALF, :] = (
            res.results[k]["out"].astype(np.float32)
        )
    return out, res


def kernel(x: np.ndarray) -> np.ndarray:
    x = np.ascontiguousarray(x, dtype=np.float32)
    assert x.shape == (B, C, H, W)
    return _run(x)[0]
